# revision 16
# baseline (speedup 1.0000x reference)
"""MoE router (top-2 of 8 experts) on 8 Trainium2 NeuronCores.

Reference computation (per token row of x [16384, 4096], W [8, 4096]):
    logits = x @ W.T                      (fp32)
    top2 values/indices, softmax(top2)    -> expert_weights [N,2], expert_indices [N,2]
    gates = softmax(logits, axis=-1); P = gates.mean(0)
    f = onehot(top1).mean(0); aux = 0.01 * 8 * sum(f*P)

Sharding: data-parallel on the token dim, 2048 tokens/core. The host passes
x shards pre-transposed ([4096, 2048]) so the contraction dim lands on SBUF
partitions with fully contiguous DMA; W is replicated (packed per-chunk).
Each core returns its ew/idx rows plus per-core sums of gates and top-1
counts; the host combines those 8x(8+8) numbers into the aux scalar.
"""

import numpy as np

import concourse.bacc as bacc
import concourse.bass as bass
import concourse.tile as tile
from concourse import mybir
from concourse.bass_utils import run_bass_kernel_spmd

N_TOK = 16384
D = 4096
E = 8
N_CORES = 8
TPC = N_TOK // N_CORES          # 2048 tokens per core
G = 2                           # token groups per core
TOK_G = TPC // G                # 1024 tokens per group
CH = D // 128                   # 32 contraction chunks
TILES_G = TOK_G // 128          # 8 token tiles per group
F32 = mybir.dt.float32
BF16 = mybir.dt.bfloat16
I32 = mybir.dt.int32
U32 = mybir.dt.uint32


def build_program(
    post: bool = True, matmul: bool = True, reps: int = 1
) -> bass.Bass:
    # Bacc (not raw Bass): its compile() runs generate_event_semaphores,
    # which splits multi-wait sync onto separate instructions — the TRN2
    # walrus here accepts at most one wait per instruction.
    nc = bacc.Bacc("TRN2", target_bir_lowering=False, debug=False)

    # x shard, host-transposed and split into bf16 hi/lo halves (x = xh + xl)
    xh_d = nc.dram_tensor("xh", [D, TPC], BF16, kind="ExternalInput")
    xl_d = nc.dram_tensor("xl", [D, TPC], BF16, kind="ExternalInput")
    # W^T packed per chunk: [p, c, {hi,lo}, e] -> [128, CH*2*E]
    wt_d = nc.dram_tensor("wt", [128, CH * 2 * E], BF16, kind="ExternalInput")
    ew_d = nc.dram_tensor("ew", [TPC, 2], F32, kind="ExternalOutput")
    idx_d = nc.dram_tensor("idx", [TPC, 2], I32, kind="ExternalOutput")
    pf_d = nc.dram_tensor("pf", [E, 2], F32, kind="ExternalOutput")

    iota_np = np.broadcast_to(np.arange(E, dtype=np.float32), (128, E)).copy()
    ones_np = np.ones((128, 1), np.float32)
    id8_np = np.eye(E, dtype=np.float32)
    iota_c = nc.inline_tensor(iota_np, name="iota_c")
    ones_c = nc.inline_tensor(ones_np, name="ones_c")
    id8_c = nc.inline_tensor(id8_np, name="id8_c")

    Exp = mybir.ActivationFunctionType.Exp
    Alu = mybir.AluOpType

    with tile.TileContext(nc) as tc:
        with (
            tc.tile_pool(name="consts", bufs=1) as cpool,
            tc.tile_pool(name="xin", bufs=6) as xpool,
            tc.tile_pool(name="lgp", bufs=1, space="PSUM") as lgpool,
            tc.tile_pool(name="lsb", bufs=2) as lsbpool,
            tc.tile_pool(name="tp", bufs=2, space="PSUM") as tppool,
            tc.tile_pool(name="pf", bufs=2, space="PSUM") as pfpool,
            tc.tile_pool(name="sm", bufs=3) as sm,
            tc.tile_pool(name="outp", bufs=4) as outp,
            tc.tile_pool(name="fin", bufs=1) as fin,
        ):
            wt = cpool.tile([128, CH * 2 * E], BF16)
            nc.sync.dma_start(wt[:], wt_d.ap())
            iota = cpool.tile([128, E], F32)
            nc.sync.dma_start(iota[:], iota_c.ap())
            ones = cpool.tile([128, 1], F32)
            nc.sync.dma_start(ones[:], ones_c.ap())
            id8 = cpool.tile([E, E], F32)
            nc.sync.dma_start(id8[:], id8_c.ap())

            def _body():
                pf_run = fin.tile([E, 2], F32)
                _run_groups(pf_run)
                nc.sync.dma_start(pf_d.ap(), pf_run[:])

            def _run_groups(pf_run):
              for g in range(G):
                # logits^T [E, TOK_G] accumulated over the 32 d-chunks
                if matmul:
                    lgT = lgpool.tile([E, TOK_G], F32)
                for c in range(CH):
                    xht = xpool.tile([128, TOK_G], BF16, tag="xht")
                    nc.sync.dma_start(
                        xht[:], xh_d.ap()[c * 128 : (c + 1) * 128,
                                          g * TOK_G : (g + 1) * TOK_G]
                    )
                    xlt = xpool.tile([128, TOK_G], BF16, tag="xlt")
                    nc.sync.dma_start(
                        xlt[:], xl_d.ap()[c * 128 : (c + 1) * 128,
                                          g * TOK_G : (g + 1) * TOK_G]
                    )
                    if not matmul:
                        nc.vector.tensor_copy(pf_run[:, g % 2 : g % 2 + 1],
                                              xht[0:E, 0:1])
                        continue
                    wh = wt[:, c * 2 * E : c * 2 * E + E]
                    wl = wt[:, c * 2 * E + E : (c + 1) * 2 * E]
                    # logits += xh@Wh + xh@Wl + xl@Wh  (xl@Wl ~ 2^-18, dropped)
                    for h in range(TOK_G // 512):
                        sl = slice(h * 512, (h + 1) * 512)
                        for k, (w_ap, x_ap) in enumerate(
                            [(wh, xht), (wl, xht), (wh, xlt)]
                        ):
                            nc.tensor.matmul(
                                lgT[:, sl],
                                lhsT=w_ap,
                                rhs=x_ap[:, sl],
                                start=(c == 0 and k == 0),
                                stop=(c == CH - 1 and k == 2),
                            )
                if not matmul:
                    continue
                if not post:
                    lgT_sb = lsbpool.tile([E, TOK_G], F32)
                    nc.vector.tensor_copy(lgT_sb[:], lgT[:])
                    nc.vector.tensor_copy(pf_run[:], lgT_sb[:, 0:2])
                    continue
                lgT_sb = lsbpool.tile([E, TOK_G], F32)
                nc.vector.tensor_copy(lgT_sb[:], lgT[:])

                pP = pfpool.tile([E, 1], F32)
                pF = pfpool.tile([E, 1], F32)
                for t in range(TILES_G):
                    lt_ps = tppool.tile([128, E], F32)
                    nc.tensor.transpose(
                        lt_ps[:], lgT_sb[:, t * 128 : (t + 1) * 128], id8[:]
                    )
                    L = sm.tile([128, E], F32)
                    nc.vector.tensor_copy(L[:], lt_ps[:])

                    maxv = sm.tile([128, E], F32)
                    nc.vector.max(maxv[:], L[:])
                    idxv = sm.tile([128, E], U32)
                    nc.vector.max_index(idxv[:], maxv[:], L[:])

                    # expert weights: softmax over the top-2 values
                    delta = sm.tile([128, 1], F32)
                    nc.vector.tensor_tensor(
                        delta[:], maxv[:, 1:2], maxv[:, 0:1], Alu.subtract
                    )
                    e2 = sm.tile([128, 1], F32)
                    nc.scalar.activation(e2[:], delta[:], Exp)
                    den = sm.tile([128, 1], F32)
                    nc.vector.tensor_scalar_add(den[:], e2[:], 1.0)
                    ew_o = outp.tile([128, 2], F32)
                    nc.vector.reciprocal(ew_o[:, 0:1], den[:])
                    nc.vector.tensor_tensor(
                        ew_o[:, 1:2], e2[:], ew_o[:, 0:1], Alu.mult
                    )

                    # full softmax over all 8 experts (for P)
                    negm = sm.tile([128, 1], F32)
                    nc.vector.tensor_scalar_mul(negm[:], maxv[:, 0:1], -1.0)
                    g8 = sm.tile([128, E], F32)
                    ssum = sm.tile([128, 1], F32)
                    nc.scalar.activation(
                        g8[:], L[:], Exp, bias=negm[:, 0:1], accum_out=ssum[:]
                    )
                    rs = sm.tile([128, 1], F32)
                    nc.vector.reciprocal(rs[:], ssum[:])
                    gates = sm.tile([128, E], F32)
                    nc.vector.tensor_scalar_mul(gates[:], g8[:], rs[:, 0:1])

                    # one-hot of the top-1 index (for f)
                    idx1f = sm.tile([128, 1], F32)
                    nc.vector.tensor_copy(idx1f[:], idxv[:, 0:1])
                    sel1 = sm.tile([128, E], F32)
                    nc.vector.tensor_scalar(
                        sel1[:], iota[:], idx1f[:, 0:1], None, Alu.is_equal
                    )

                    # per-core sums over tokens via ones-matmul
                    nc.tensor.matmul(
                        pP[:], lhsT=gates[:], rhs=ones[:],
                        start=(t == 0), stop=(t == TILES_G - 1),
                    )
                    nc.tensor.matmul(
                        pF[:], lhsT=sel1[:], rhs=ones[:],
                        start=(t == 0), stop=(t == TILES_G - 1),
                    )

                    row = g * TOK_G + t * 128
                    nc.sync.dma_start(ew_d.ap()[row : row + 128, :], ew_o[:])
                    nc.sync.dma_start(
                        idx_d.ap()[row : row + 128, :],
                        idxv[:, 0:2].bitcast(I32),
                    )

                if g == 0:
                    nc.vector.tensor_copy(pf_run[:, 0:1], pP[:])
                    nc.vector.tensor_copy(pf_run[:, 1:2], pF[:])
                else:
                    nc.vector.tensor_tensor(
                        pf_run[:, 0:1], pf_run[:, 0:1], pP[:], Alu.add
                    )
                    nc.vector.tensor_tensor(
                        pf_run[:, 1:2], pf_run[:, 1:2], pF[:], Alu.add
                    )
            if reps == 1:
                _body()
            else:
                with tc.For_i(
                    0, reps, 1,
                    hint_engines=(
                        mybir.EngineType.PE,
                        mybir.EngineType.DVE,
                        mybir.EngineType.Activation,
                        mybir.EngineType.SP,
                    ),
                ):
                    _body()

    nc.compile()
    return nc


_PROGRAM = None


def _get_program() -> bass.Bass:
    global _PROGRAM
    if _PROGRAM is None:
        _PROGRAM = build_program()
    return _PROGRAM


def _split_bf16(a: np.ndarray) -> tuple[np.ndarray, np.ndarray]:
    import ml_dtypes

    hi = a.astype(ml_dtypes.bfloat16)
    lo = (a - hi.astype(np.float32)).astype(ml_dtypes.bfloat16)
    return hi, lo


def make_in_maps(x: np.ndarray, W: np.ndarray) -> list[dict[str, np.ndarray]]:
    x = np.asarray(x, np.float32)
    W = np.asarray(W, np.float32)
    # wt[p, c, {hi,lo}, e] = split(W[e, c*128+p])
    wtf = W.T.reshape(CH, 128, E).transpose(1, 0, 2)  # [128, CH, E] fp32
    wh, wl = _split_bf16(wtf)
    wt = np.ascontiguousarray(
        np.stack([wh, wl], axis=2).reshape(128, CH * 2 * E)
    )
    xt = np.ascontiguousarray(x.T)                    # [D, N] fp32
    xh, xl = _split_bf16(xt)
    maps = []
    for c in range(N_CORES):
        sl = slice(c * TPC, (c + 1) * TPC)
        maps.append({
            "xh": np.ascontiguousarray(xh[:, sl]),
            "xl": np.ascontiguousarray(xl[:, sl]),
            "wt": wt,
        })
    return maps


def combine_outputs(results: list[dict[str, np.ndarray]]):
    ew = np.concatenate([results[c]["ew"] for c in range(N_CORES)], axis=0)
    idx = np.concatenate([results[c]["idx"] for c in range(N_CORES)], axis=0)
    idx = idx.astype(np.int32)
    pf = np.stack([results[c]["pf"] for c in range(N_CORES)])  # [cores, 8, 2]
    P = pf[:, :, 0].sum(axis=0, dtype=np.float32) / np.float32(N_TOK)
    F = pf[:, :, 1].sum(axis=0, dtype=np.float32) / np.float32(N_TOK)
    aux = np.float32(0.01 * E * np.sum(F * P, dtype=np.float32))
    return ew, idx, aux


def kernel(x: np.ndarray, W: np.ndarray):
    nc = _get_program()
    in_maps = make_in_maps(x, W)
    res = run_bass_kernel_spmd(nc, in_maps, list(range(N_CORES)))
    return combine_outputs(res.results)


# revision 24
# speedup vs baseline: 50.0510x; 50.0510x over previous
"""MoE router (top-2 of 8 experts) on 8 Trainium2 NeuronCores.

Reference computation (per token row of x [16384, 4096], W [8, 4096]):
    logits = x @ W.T                      (fp32)
    top2 values/indices, softmax(top2)    -> expert_weights [N,2], expert_indices [N,2]
    gates = softmax(logits, axis=-1); P = gates.mean(0)
    f = onehot(top1).mean(0); aux = 0.01 * 8 * sum(f*P)

Sharding: data-parallel on the token dim, 2048 tokens/core. The host passes
x shards pre-transposed ([4096, 2048]) so the contraction dim lands on SBUF
partitions with fully contiguous DMA; W is replicated (packed per-chunk).
Each core returns its ew/idx rows plus per-core sums of gates and top-1
counts; the host combines those 8x(8+8) numbers into the aux scalar.
"""

import numpy as np

import concourse.bacc as bacc
import concourse.bass as bass
import concourse.tile as tile
from concourse import mybir
from concourse.bass_utils import run_bass_kernel_spmd

N_TOK = 16384
D = 4096
E = 8
N_CORES = 8
TPC = N_TOK // N_CORES          # 2048 tokens per core
G = 2                           # token groups per core
TOK_G = TPC // G                # 1024 tokens per group
CH = D // 128                   # 32 contraction chunks
TILES_G = TOK_G // 128          # 8 token tiles per group
F32 = mybir.dt.float32
BF16 = mybir.dt.bfloat16
I32 = mybir.dt.int32
U32 = mybir.dt.uint32


def build_program(
    post: bool = True, matmul: bool = True, reps: int = 1
) -> bass.Bass:
    # Bacc (not raw Bass): its compile() runs generate_event_semaphores,
    # which splits multi-wait sync onto separate instructions — the TRN2
    # walrus here accepts at most one wait per instruction.
    nc = bacc.Bacc("TRN2", target_bir_lowering=False, debug=False)

    # x shard, host-transposed and split into bf16 hi/lo halves (x = xh + xl)
    xh_d = nc.dram_tensor("xh", [D, TPC], BF16, kind="ExternalInput")
    xl_d = nc.dram_tensor("xl", [D, TPC], BF16, kind="ExternalInput")
    # W^T packed per chunk: [p, c, {hi,lo}, e] -> [128, CH*2*E]
    wt_d = nc.dram_tensor("wt", [128, CH * 2 * E], BF16, kind="ExternalInput")
    # outputs in partition-major staging layout: [p, g*TILES_G+t, k] holds
    # token g*TOK_G + t*128 + p; host unpermutes (keeps DMA descriptors big)
    TT = G * TILES_G
    ew_d = nc.dram_tensor("ewt", [128, TT * 2], F32, kind="ExternalOutput")
    idx_d = nc.dram_tensor("idxt", [128, TT * 2], I32, kind="ExternalOutput")
    pf_d = nc.dram_tensor("pf", [E, 2], F32, kind="ExternalOutput")

    iota_np = np.broadcast_to(np.arange(E, dtype=np.float32), (128, E)).copy()
    ones_np = np.ones((128, 1), np.float32)
    id8_np = np.eye(E, dtype=np.float32)
    iota_c = nc.inline_tensor(iota_np, name="iota_c")
    ones_c = nc.inline_tensor(ones_np, name="ones_c")
    id8_c = nc.inline_tensor(id8_np, name="id8_c")

    Exp = mybir.ActivationFunctionType.Exp
    Alu = mybir.AluOpType

    with tile.TileContext(nc) as tc:
        with (
            tc.tile_pool(name="consts", bufs=1) as cpool,
            tc.tile_pool(name="xin", bufs=6) as xpool,
            tc.tile_pool(name="lgp", bufs=1, space="PSUM") as lgpool,
            tc.tile_pool(name="lsb", bufs=2) as lsbpool,
            tc.tile_pool(name="tp", bufs=2, space="PSUM") as tppool,
            tc.tile_pool(name="pf", bufs=2, space="PSUM") as pfpool,
            tc.tile_pool(name="sm", bufs=3) as sm,
            tc.tile_pool(name="fin", bufs=1) as fin,
        ):
            wt = cpool.tile([128, CH * 2 * E], BF16)
            nc.sync.dma_start(wt[:], wt_d.ap())
            iota = cpool.tile([128, E], F32)
            nc.sync.dma_start(iota[:], iota_c.ap())
            ones = cpool.tile([128, 1], F32)
            nc.sync.dma_start(ones[:], ones_c.ap())
            id8 = cpool.tile([E, E], F32)
            nc.sync.dma_start(id8[:], id8_c.ap())

            def _body():
                pf_run = fin.tile([E, 2], F32)
                ew_stage = fin.tile([128, TT, 2], F32)
                idx_stage = fin.tile([128, TT, 2], I32)
                _run_groups(pf_run, ew_stage, idx_stage)
                nc.sync.dma_start(pf_d.ap(), pf_run[:])
                if post and matmul:
                    nc.sync.dma_start(ew_d.ap(), ew_stage[:])
                    nc.sync.dma_start(idx_d.ap(), idx_stage[:])

            def _run_groups(pf_run, ew_stage, idx_stage):
              for g in range(G):
                # logits^T [E, TOK_G] accumulated over the 32 d-chunks
                if matmul:
                    lgT = lgpool.tile([E, TOK_G], F32)
                for c in range(CH):
                    xht = xpool.tile([128, TOK_G], BF16, tag="xht")
                    nc.sync.dma_start(
                        xht[:], xh_d.ap()[c * 128 : (c + 1) * 128,
                                          g * TOK_G : (g + 1) * TOK_G]
                    )
                    xlt = xpool.tile([128, TOK_G], BF16, tag="xlt")
                    nc.sync.dma_start(
                        xlt[:], xl_d.ap()[c * 128 : (c + 1) * 128,
                                          g * TOK_G : (g + 1) * TOK_G]
                    )
                    if not matmul:
                        nc.vector.tensor_copy(pf_run[:, g % 2 : g % 2 + 1],
                                              xht[0:E, 0:1])
                        continue
                    wh = wt[:, c * 2 * E : c * 2 * E + E]
                    wl = wt[:, c * 2 * E + E : (c + 1) * 2 * E]
                    # logits += xh@Wh + xh@Wl + xl@Wh  (xl@Wl ~ 2^-18, dropped)
                    for h in range(TOK_G // 512):
                        sl = slice(h * 512, (h + 1) * 512)
                        for k, (w_ap, x_ap) in enumerate(
                            [(wh, xht), (wl, xht), (wh, xlt)]
                        ):
                            nc.tensor.matmul(
                                lgT[:, sl],
                                lhsT=w_ap,
                                rhs=x_ap[:, sl],
                                start=(c == 0 and k == 0),
                                stop=(c == CH - 1 and k == 2),
                            )
                if not matmul:
                    continue
                if not post:
                    lgT_sb = lsbpool.tile([E, TOK_G], F32)
                    nc.vector.tensor_copy(lgT_sb[:], lgT[:])
                    nc.vector.tensor_copy(pf_run[:], lgT_sb[:, 0:2])
                    continue
                lgT_sb = lsbpool.tile([E, TOK_G], F32)
                nc.vector.tensor_copy(lgT_sb[:], lgT[:])

                # transpose the group's logits into one [128, TILES_G, E] tile
                ltg = tppool.tile([128, TILES_G, E], F32)
                for t in range(TILES_G):
                    nc.tensor.transpose(
                        ltg[:, t : t + 1, :],
                        lgT_sb[:, t * 128 : (t + 1) * 128],
                        id8[:],
                    )
                La = sm.tile([128, TILES_G, E], F32)
                nc.vector.tensor_copy(La[:], ltg[:])

                # top-8 sort + indices per token tile (per-tile HW op)
                maxv = sm.tile([128, TILES_G, E], F32)
                idxv = sm.tile([128, TILES_G, E], U32)
                for t in range(TILES_G):
                    nc.vector.max(
                        maxv[:, t : t + 1, :].squeeze(1),
                        La[:, t : t + 1, :].squeeze(1),
                    )
                    nc.vector.max_index(
                        idxv[:, t : t + 1, :].squeeze(1),
                        maxv[:, t : t + 1, :].squeeze(1),
                        La[:, t : t + 1, :].squeeze(1),
                    )

                gsl = slice(g * TILES_G, (g + 1) * TILES_G)
                # expert weights: softmax over the top-2 values (batched)
                delta = sm.tile([128, TILES_G, 1], F32)
                nc.vector.tensor_tensor(
                    delta[:], maxv[:, :, 1:2], maxv[:, :, 0:1], Alu.subtract
                )
                e2 = sm.tile([128, TILES_G, 1], F32)
                nc.scalar.activation(e2[:], delta[:], Exp)
                den = sm.tile([128, TILES_G, 1], F32)
                nc.vector.tensor_scalar_add(den[:], e2[:], 1.0)
                nc.vector.reciprocal(ew_stage[:, gsl, 0:1], den[:])
                nc.vector.tensor_tensor(
                    ew_stage[:, gsl, 1:2], e2[:], ew_stage[:, gsl, 0:1],
                    Alu.mult,
                )
                nc.vector.tensor_copy(
                    idx_stage[:, gsl, :], idxv[:, :, 0:2].bitcast(I32)
                )

                # full softmax over all 8 experts (for P), batched
                xm = sm.tile([128, TILES_G, E], F32)
                nc.vector.tensor_tensor(
                    xm[:], La[:],
                    maxv[:, :, 0:1].broadcast_to([128, TILES_G, E]),
                    Alu.subtract,
                )
                g64 = sm.tile([128, TILES_G, E], F32)
                nc.scalar.activation(g64[:], xm[:], Exp)
                ssum = sm.tile([128, TILES_G, 1], F32)
                nc.vector.tensor_reduce(
                    ssum[:], g64[:], mybir.AxisListType.X, Alu.add
                )
                rs = sm.tile([128, TILES_G, 1], F32)
                nc.vector.reciprocal(rs[:], ssum[:])
                gates = sm.tile([128, TILES_G, E], F32)
                nc.vector.tensor_tensor(
                    gates[:], g64[:],
                    rs[:].broadcast_to([128, TILES_G, E]),
                    Alu.mult,
                )

                # one-hot of the top-1 index (for f), batched
                idx1f = sm.tile([128, TILES_G, 1], F32)
                nc.vector.tensor_copy(idx1f[:], idxv[:, :, 0:1])
                sel1 = sm.tile([128, TILES_G, E], F32)
                nc.vector.tensor_tensor(
                    sel1[:],
                    iota[:].unsqueeze(1).broadcast_to([128, TILES_G, E]),
                    idx1f[:].broadcast_to([128, TILES_G, E]),
                    Alu.is_equal,
                )

                # per-core sums over tokens via ones-matmul
                pP = pfpool.tile([E, 1], F32)
                pF = pfpool.tile([E, 1], F32)
                for t in range(TILES_G):
                    nc.tensor.matmul(
                        pP[:], lhsT=gates[:, t : t + 1, :], rhs=ones[:],
                        start=(t == 0), stop=(t == TILES_G - 1),
                    )
                    nc.tensor.matmul(
                        pF[:], lhsT=sel1[:, t : t + 1, :], rhs=ones[:],
                        start=(t == 0), stop=(t == TILES_G - 1),
                    )

                if g == 0:
                    nc.vector.tensor_copy(pf_run[:, 0:1], pP[:])
                    nc.vector.tensor_copy(pf_run[:, 1:2], pF[:])
                else:
                    nc.vector.tensor_tensor(
                        pf_run[:, 0:1], pf_run[:, 0:1], pP[:], Alu.add
                    )
                    nc.vector.tensor_tensor(
                        pf_run[:, 1:2], pf_run[:, 1:2], pF[:], Alu.add
                    )
            if reps == 1:
                _body()
            else:
                with tc.For_i(
                    0, reps, 1,
                    hint_engines=(
                        mybir.EngineType.PE,
                        mybir.EngineType.DVE,
                        mybir.EngineType.Activation,
                        mybir.EngineType.SP,
                    ),
                ):
                    _body()

    nc.compile()
    return nc


_PROGRAM = None


def _get_program() -> bass.Bass:
    global _PROGRAM
    if _PROGRAM is None:
        _PROGRAM = build_program()
    return _PROGRAM


def _split_bf16(a: np.ndarray) -> tuple[np.ndarray, np.ndarray]:
    import ml_dtypes

    hi = a.astype(ml_dtypes.bfloat16)
    lo = (a - hi.astype(np.float32)).astype(ml_dtypes.bfloat16)
    return hi, lo


def make_in_maps(x: np.ndarray, W: np.ndarray) -> list[dict[str, np.ndarray]]:
    x = np.asarray(x, np.float32)
    W = np.asarray(W, np.float32)
    # wt[p, c, {hi,lo}, e] = split(W[e, c*128+p])
    wtf = W.T.reshape(CH, 128, E).transpose(1, 0, 2)  # [128, CH, E] fp32
    wh, wl = _split_bf16(wtf)
    wt = np.ascontiguousarray(
        np.stack([wh, wl], axis=2).reshape(128, CH * 2 * E)
    )
    xt = np.ascontiguousarray(x.T)                    # [D, N] fp32
    xh, xl = _split_bf16(xt)
    maps = []
    for c in range(N_CORES):
        sl = slice(c * TPC, (c + 1) * TPC)
        maps.append({
            "xh": np.ascontiguousarray(xh[:, sl]),
            "xl": np.ascontiguousarray(xl[:, sl]),
            "wt": wt,
        })
    return maps


def _unstage(a: np.ndarray) -> np.ndarray:
    # [128, TT*2] staging -> [TPC, 2] token-major
    return (
        a.reshape(128, G * TILES_G, 2).transpose(1, 0, 2).reshape(TPC, 2)
    )


def combine_outputs(results: list[dict[str, np.ndarray]]):
    ew = np.concatenate(
        [_unstage(results[c]["ewt"]) for c in range(N_CORES)], axis=0
    )
    idx = np.concatenate(
        [_unstage(results[c]["idxt"]) for c in range(N_CORES)], axis=0
    )
    idx = idx.astype(np.int32)
    pf = np.stack([results[c]["pf"] for c in range(N_CORES)])  # [cores, 8, 2]
    P = pf[:, :, 0].sum(axis=0, dtype=np.float32) / np.float32(N_TOK)
    F = pf[:, :, 1].sum(axis=0, dtype=np.float32) / np.float32(N_TOK)
    aux = np.float32(0.01 * E * np.sum(F * P, dtype=np.float32))
    return ew, idx, aux


def kernel(x: np.ndarray, W: np.ndarray):
    nc = _get_program()
    in_maps = make_in_maps(x, W)
    res = run_bass_kernel_spmd(nc, in_maps, list(range(N_CORES)))
    return combine_outputs(res.results)


# revision 30
# speedup vs baseline: 62.1914x; 1.2426x over previous
"""MoE router (top-2 of 8 experts) on 8 Trainium2 NeuronCores.

Reference computation (per token row of x [16384, 4096], W [8, 4096]):
    logits = x @ W.T                      (fp32)
    top2 values/indices, softmax(top2)    -> expert_weights [N,2], expert_indices [N,2]
    gates = softmax(logits, axis=-1); P = gates.mean(0)
    f = onehot(top1).mean(0); aux = 0.01 * 8 * sum(f*P)

Sharding: data-parallel on the token dim, 2048 tokens/core. The host passes
x shards pre-transposed ([4096, 2048]) so the contraction dim lands on SBUF
partitions with fully contiguous DMA; W is replicated (packed per-chunk).
Each core returns its ew/idx rows plus per-core sums of gates and top-1
counts; the host combines those 8x(8+8) numbers into the aux scalar.
"""

import numpy as np

import concourse.bacc as bacc
import concourse.bass as bass
import concourse.tile as tile
from concourse import mybir
from concourse.bass_utils import run_bass_kernel_spmd

N_TOK = 16384
D = 4096
E = 8
N_CORES = 8
TPC = N_TOK // N_CORES          # 2048 tokens per core
G = 2                           # token groups per core
TOK_G = TPC // G                # 1024 tokens per group
CH = D // 128                   # 32 contraction chunks
PACK = 4                        # d-chunks fetched per DMA (8 KiB/descriptor)
SUP = CH // PACK                # superblocks per group
TILES_G = TOK_G // 128          # 8 token tiles per group
F32 = mybir.dt.float32
BF16 = mybir.dt.bfloat16
I32 = mybir.dt.int32
U32 = mybir.dt.uint32


def build_program(
    post: bool = True, matmul: bool = True, reps: int = 1
) -> bass.Bass:
    # Bacc (not raw Bass): its compile() runs generate_event_semaphores,
    # which splits multi-wait sync onto separate instructions — the TRN2
    # walrus here accepts at most one wait per instruction.
    nc = bacc.Bacc("TRN2", target_bir_lowering=False, debug=False)

    # x shard, host-transposed, split into bf16 hi/lo halves (x = xh + xl),
    # and packed so each DMA reads PACK chunk-rows contiguously per partition:
    # [g*SUP + sup, p, sub*TOK_G + tok] = x[g*TOK_G+tok, (sup*PACK+sub)*128+p]
    xh_d = nc.dram_tensor(
        "xh", [G * SUP, 128, PACK * TOK_G], BF16, kind="ExternalInput"
    )
    xl_d = nc.dram_tensor(
        "xl", [G * SUP, 128, PACK * TOK_G], BF16, kind="ExternalInput"
    )
    # W^T packed per chunk: [p, c, {hi,lo}, e] -> [128, CH*2*E]
    wt_d = nc.dram_tensor("wt", [128, CH * 2 * E], BF16, kind="ExternalInput")
    # outputs in partition-major staging layout: [p, g*TILES_G+t, k] holds
    # token g*TOK_G + t*128 + p; host unpermutes (keeps DMA descriptors big)
    TT = G * TILES_G
    ew_d = nc.dram_tensor("ewt", [128, TT * 2], F32, kind="ExternalOutput")
    idx_d = nc.dram_tensor("idxt", [128, TT * 2], I32, kind="ExternalOutput")
    pf_d = nc.dram_tensor("pf", [E, 2], F32, kind="ExternalOutput")

    iota_np = np.broadcast_to(np.arange(E, dtype=np.float32), (128, E)).copy()
    ones_np = np.ones((128, 1), np.float32)
    id8_np = np.eye(E, dtype=np.float32)
    iota_c = nc.inline_tensor(iota_np, name="iota_c")
    ones_c = nc.inline_tensor(ones_np, name="ones_c")
    id8_c = nc.inline_tensor(id8_np, name="id8_c")

    Exp = mybir.ActivationFunctionType.Exp
    Alu = mybir.AluOpType

    with tile.TileContext(nc) as tc:
        with (
            tc.tile_pool(name="consts", bufs=1) as cpool,
            tc.tile_pool(name="xin", bufs=6) as xpool,
            tc.tile_pool(name="lgp", bufs=1, space="PSUM") as lgpool,
            tc.tile_pool(name="lsb", bufs=2) as lsbpool,
            tc.tile_pool(name="tp", bufs=2, space="PSUM") as tppool,
            tc.tile_pool(name="pf", bufs=2, space="PSUM") as pfpool,
            tc.tile_pool(name="sm", bufs=3) as sm,
            tc.tile_pool(name="fin", bufs=1) as fin,
        ):
            wt = cpool.tile([128, CH * 2 * E], BF16)
            nc.sync.dma_start(wt[:], wt_d.ap())
            iota = cpool.tile([128, E], F32)
            nc.sync.dma_start(iota[:], iota_c.ap())
            ones = cpool.tile([128, 1], F32)
            nc.sync.dma_start(ones[:], ones_c.ap())
            id8 = cpool.tile([E, E], F32)
            nc.sync.dma_start(id8[:], id8_c.ap())

            def _body():
                pf_run = fin.tile([E, 2], F32)
                ew_stage = fin.tile([128, TT, 2], F32)
                idx_stage = fin.tile([128, TT, 2], I32)
                _run_groups(pf_run, ew_stage, idx_stage)
                nc.sync.dma_start(pf_d.ap(), pf_run[:])
                if post and matmul:
                    nc.sync.dma_start(ew_d.ap(), ew_stage[:])
                    nc.sync.dma_start(idx_d.ap(), idx_stage[:])

            def _run_groups(pf_run, ew_stage, idx_stage):
              for g in range(G):
                # logits^T [E, TOK_G] accumulated over the 32 d-chunks
                if matmul:
                    lgT = lgpool.tile([E, TOK_G], F32)
                for sup in range(SUP):
                    xht = xpool.tile([128, PACK, TOK_G], BF16, tag="xht")
                    nc.sync.dma_start(xht[:], xh_d.ap()[g * SUP + sup])
                    xlt = xpool.tile([128, PACK, TOK_G], BF16, tag="xlt")
                    nc.sync.dma_start(xlt[:], xl_d.ap()[g * SUP + sup])
                    if not matmul:
                        nc.vector.tensor_copy(pf_run[:, g % 2 : g % 2 + 1],
                                              xht[0:E, 0:1, 0:1])
                        continue
                    for sub in range(PACK):
                        c = sup * PACK + sub
                        wh = wt[:, c * 2 * E : c * 2 * E + E]
                        wl = wt[:, c * 2 * E + E : (c + 1) * 2 * E]
                        # logits += xh@Wh + xh@Wl + xl@Wh (xl@Wl ~2^-18, drop)
                        for h in range(TOK_G // 512):
                            sl = slice(h * 512, (h + 1) * 512)
                            for k, (w_ap, x_ap) in enumerate(
                                [(wh, xht), (wl, xht), (wh, xlt)]
                            ):
                                nc.tensor.matmul(
                                    lgT[:, sl],
                                    lhsT=w_ap,
                                    rhs=x_ap[:, sub : sub + 1, sl],
                                    start=(c == 0 and k == 0),
                                    stop=(c == CH - 1 and k == 2),
                                )
                if not matmul:
                    continue
                if not post:
                    lgT_sb = lsbpool.tile([E, TOK_G], F32)
                    nc.vector.tensor_copy(lgT_sb[:], lgT[:])
                    nc.vector.tensor_copy(pf_run[:], lgT_sb[:, 0:2])
                    continue
                lgT_sb = lsbpool.tile([E, TOK_G], F32)
                nc.vector.tensor_copy(lgT_sb[:], lgT[:])

                # transpose the group's logits into one [128, TILES_G, E] tile
                ltg = tppool.tile([128, TILES_G, E], F32)
                for t in range(TILES_G):
                    nc.tensor.transpose(
                        ltg[:, t : t + 1, :],
                        lgT_sb[:, t * 128 : (t + 1) * 128],
                        id8[:],
                    )
                La = sm.tile([128, TILES_G, E], F32)
                nc.vector.tensor_copy(La[:], ltg[:])

                # top-8 sort + indices per token tile (per-tile HW op)
                maxv = sm.tile([128, TILES_G, E], F32)
                idxv = sm.tile([128, TILES_G, E], U32)
                for t in range(TILES_G):
                    nc.vector.max(
                        maxv[:, t : t + 1, :].squeeze(1),
                        La[:, t : t + 1, :].squeeze(1),
                    )
                    nc.vector.max_index(
                        idxv[:, t : t + 1, :].squeeze(1),
                        maxv[:, t : t + 1, :].squeeze(1),
                        La[:, t : t + 1, :].squeeze(1),
                    )

                gsl = slice(g * TILES_G, (g + 1) * TILES_G)
                # expert weights: softmax over the top-2 values (batched)
                delta = sm.tile([128, TILES_G, 1], F32)
                nc.vector.tensor_tensor(
                    delta[:], maxv[:, :, 1:2], maxv[:, :, 0:1], Alu.subtract
                )
                e2 = sm.tile([128, TILES_G, 1], F32)
                nc.scalar.activation(e2[:], delta[:], Exp)
                den = sm.tile([128, TILES_G, 1], F32)
                nc.vector.tensor_scalar_add(den[:], e2[:], 1.0)
                nc.vector.reciprocal(ew_stage[:, gsl, 0:1], den[:])
                nc.vector.tensor_tensor(
                    ew_stage[:, gsl, 1:2], e2[:], ew_stage[:, gsl, 0:1],
                    Alu.mult,
                )
                nc.vector.tensor_copy(
                    idx_stage[:, gsl, :], idxv[:, :, 0:2].bitcast(I32)
                )

                # full softmax over all 8 experts (for P), batched
                xm = sm.tile([128, TILES_G, E], F32)
                nc.vector.tensor_tensor(
                    xm[:], La[:],
                    maxv[:, :, 0:1].broadcast_to([128, TILES_G, E]),
                    Alu.subtract,
                )
                g64 = sm.tile([128, TILES_G, E], F32)
                nc.scalar.activation(g64[:], xm[:], Exp)
                ssum = sm.tile([128, TILES_G, 1], F32)
                nc.vector.tensor_reduce(
                    ssum[:], g64[:], mybir.AxisListType.X, Alu.add
                )
                rs = sm.tile([128, TILES_G, 1], F32)
                nc.vector.reciprocal(rs[:], ssum[:])
                gates = sm.tile([128, TILES_G, E], F32)
                nc.vector.tensor_tensor(
                    gates[:], g64[:],
                    rs[:].broadcast_to([128, TILES_G, E]),
                    Alu.mult,
                )

                # one-hot of the top-1 index (for f), batched
                idx1f = sm.tile([128, TILES_G, 1], F32)
                nc.vector.tensor_copy(idx1f[:], idxv[:, :, 0:1])
                sel1 = sm.tile([128, TILES_G, E], F32)
                nc.vector.tensor_tensor(
                    sel1[:],
                    iota[:].unsqueeze(1).broadcast_to([128, TILES_G, E]),
                    idx1f[:].broadcast_to([128, TILES_G, E]),
                    Alu.is_equal,
                )

                # per-core sums over tokens via ones-matmul
                pP = pfpool.tile([E, 1], F32)
                pF = pfpool.tile([E, 1], F32)
                for t in range(TILES_G):
                    nc.tensor.matmul(
                        pP[:], lhsT=gates[:, t : t + 1, :], rhs=ones[:],
                        start=(t == 0), stop=(t == TILES_G - 1),
                    )
                    nc.tensor.matmul(
                        pF[:], lhsT=sel1[:, t : t + 1, :], rhs=ones[:],
                        start=(t == 0), stop=(t == TILES_G - 1),
                    )

                if g == 0:
                    nc.vector.tensor_copy(pf_run[:, 0:1], pP[:])
                    nc.vector.tensor_copy(pf_run[:, 1:2], pF[:])
                else:
                    nc.vector.tensor_tensor(
                        pf_run[:, 0:1], pf_run[:, 0:1], pP[:], Alu.add
                    )
                    nc.vector.tensor_tensor(
                        pf_run[:, 1:2], pf_run[:, 1:2], pF[:], Alu.add
                    )
            if reps == 1:
                _body()
            else:
                with tc.For_i(
                    0, reps, 1,
                    hint_engines=(
                        mybir.EngineType.PE,
                        mybir.EngineType.DVE,
                        mybir.EngineType.Activation,
                        mybir.EngineType.SP,
                    ),
                ):
                    _body()

    nc.compile()
    return nc


_PROGRAM = None


def _get_program() -> bass.Bass:
    global _PROGRAM
    if _PROGRAM is None:
        _PROGRAM = build_program()
    return _PROGRAM


def _split_bf16(a: np.ndarray) -> tuple[np.ndarray, np.ndarray]:
    import ml_dtypes

    hi = a.astype(ml_dtypes.bfloat16)
    lo = (a - hi.astype(np.float32)).astype(ml_dtypes.bfloat16)
    return hi, lo


def make_in_maps(x: np.ndarray, W: np.ndarray) -> list[dict[str, np.ndarray]]:
    x = np.asarray(x, np.float32)
    W = np.asarray(W, np.float32)
    # wt[p, c, {hi,lo}, e] = split(W[e, c*128+p])
    wtf = W.T.reshape(CH, 128, E).transpose(1, 0, 2)  # [128, CH, E] fp32
    wh, wl = _split_bf16(wtf)
    wt = np.ascontiguousarray(
        np.stack([wh, wl], axis=2).reshape(128, CH * 2 * E)
    )
    xt = np.ascontiguousarray(x.T)                    # [D, N] fp32
    xh, xl = _split_bf16(xt)

    def _pack(a, core):
        # [D, N] -> per-core [G*SUP, 128, PACK*TOK_G]
        s = a[:, core * TPC : (core + 1) * TPC]       # [D, TPC]
        s = s.reshape(SUP, PACK, 128, G, TOK_G)
        s = s.transpose(3, 0, 2, 1, 4)                # [G, SUP, 128, PACK, TOK_G]
        return np.ascontiguousarray(s.reshape(G * SUP, 128, PACK * TOK_G))

    maps = []
    for c in range(N_CORES):
        maps.append({"xh": _pack(xh, c), "xl": _pack(xl, c), "wt": wt})
    return maps


def _unstage(a: np.ndarray) -> np.ndarray:
    # [128, TT*2] staging -> [TPC, 2] token-major
    return (
        a.reshape(128, G * TILES_G, 2).transpose(1, 0, 2).reshape(TPC, 2)
    )


def combine_outputs(results: list[dict[str, np.ndarray]]):
    ew = np.concatenate(
        [_unstage(results[c]["ewt"]) for c in range(N_CORES)], axis=0
    )
    idx = np.concatenate(
        [_unstage(results[c]["idxt"]) for c in range(N_CORES)], axis=0
    )
    idx = idx.astype(np.int32)
    pf = np.stack([results[c]["pf"] for c in range(N_CORES)])  # [cores, 8, 2]
    P = pf[:, :, 0].sum(axis=0, dtype=np.float32) / np.float32(N_TOK)
    F = pf[:, :, 1].sum(axis=0, dtype=np.float32) / np.float32(N_TOK)
    aux = np.float32(0.01 * E * np.sum(F * P, dtype=np.float32))
    return ew, idx, aux


def kernel(x: np.ndarray, W: np.ndarray):
    nc = _get_program()
    in_maps = make_in_maps(x, W)
    try:
        res = run_bass_kernel_spmd(nc, in_maps, list(range(N_CORES)))
    except Exception:
        # one retry in case a device was left in a transient bad state
        res = run_bass_kernel_spmd(nc, in_maps, list(range(N_CORES)))
    return combine_outputs(res.results)
